# revision 34
# baseline (speedup 1.0000x reference)
"""Trainium2 Bass kernel for nn_CMAlign_mask (sparse_attention).

Strategy: data-parallel over the 64-sample batch, 8 samples per NeuronCore.
The cross-sample gathers feat[pos_idx]/feat[neg_idx] are resolved on the host
during sharding (indices are host-visible), so the device program is purely
per-sample. Each sample's feature tensors are shipped in two layouts:
  - natural [f, hw] (f interleaved as 16p+c) for the gram matmul G = x_q^T x_k
  - transposed [hw, f] for the warp matmul (contraction over hw)

Matmuls run as float32r (~1e-4 rel err, ~2x faster than fp32; measured on HW).
The warp PSUM is pre-seeded with -feat via a -Identity matmul so the PSUM ends
up holding warp-feat directly; the softmax denominator is folded into the
transposed probabilities. Per-sample work is software-pipelined (softmax of
sample i-1 | loads+G of sample i | warp+consume of sample i-1) to keep the PE
stream dense.
"""

import sys

if "/opt/trn_rl_repo" not in sys.path:
    sys.path.insert(0, "/opt/trn_rl_repo")

import numpy as np

NS = 8  # samples per core
HW = 162
F = 2048
NB = 4  # f blocks of 512
TEMP = 50.0
MARGIN = 0.3
EPS_PD = 1e-6
QS = [(0, 128), (128, 34)]  # hw splits (partition-dim limit)

TRACE = False  # set by test harness for profiling
LAST_EXEC_NS = None

_cache = {}


def build_program():
    import concourse.bass as bass  # noqa: F401
    import concourse.mybir as mybir
    import concourse.tile as tile
    from concourse import bacc
    from concourse.masks import make_identity

    f32 = mybir.dt.float32
    f32r = mybir.dt.float32r
    AF = mybir.ActivationFunctionType
    OP = mybir.AluOpType
    AX = mybir.AxisListType

    nc = bacc.Bacc("TRN2", target_bir_lowering=False, debug=False, num_devices=8)

    xqT_d = nc.dram_tensor("xqT", [NS, HW, F], f32r, kind="ExternalInput").ap()
    xqn_d = nc.dram_tensor(
        "xqnat", [NS, 128, 16, HW], f32r, kind="ExternalInput"
    ).ap()
    rhs_d = nc.dram_tensor("rhs", [NS, 128, 16, 324], f32r, kind="ExternalInput").ap()
    tpn_d = nc.dram_tensor("tpn", [NS, HW, 2, F], f32r, kind="ExternalInput").ap()
    textT_d = nc.dram_tensor("textT", [512, 2 * NS], f32, kind="ExternalInput").ap()
    mwT_d = nc.dram_tensor("mwT", [512, HW], f32, kind="ExternalInput").ap()
    reconT_d = nc.dram_tensor("reconT", [NS, HW, F], f32, kind="ExternalOutput").ap()
    loss_d = nc.dram_tensor("loss", [1, 1], f32, kind="ExternalOutput").ap()

    with tile.TileContext(nc) as tc:
        with (
            tc.tile_pool(name="persist", bufs=1) as P,
            tc.tile_pool(name="io2", bufs=2) as IO,
            tc.tile_pool(name="io1", bufs=1) as IO1,
            tc.tile_pool(name="wk", bufs=2) as WK,
            tc.tile_pool(name="ps_mm", bufs=2, space="PSUM") as PSM,
            tc.tile_pool(name="ps_w", bufs=3, space="PSUM") as PSW,
            tc.tile_pool(name="dram", bufs=2, space="DRAM") as DR,
        ):
            ident = P.tile([128, 128], f32)
            make_identity(nc, ident)
            negI = P.tile([128, 128], f32r)
            nc.scalar.mul(negI, ident, -1.0)
            cm50 = P.tile([128, 1], f32)
            nc.vector.memset(cm50, -TEMP)
            ceps = P.tile([128, 1], f32)
            nc.vector.memset(ceps, EPS_PD)
            dots = P.tile([1, NS], f32)

            # ---------------- mask preamble ----------------
            textT_s = P.tile([128, 4, 2 * NS], f32)
            nc.sync.dma_start(
                out=textT_s, in_=textT_d.rearrange("(c p) j -> p c j", p=128)
            )
            mwT_s = P.tile([128, 4, HW], f32)
            nc.sync.dma_start(out=mwT_s, in_=mwT_d.rearrange("(c p) s -> p c s", p=128))
            nm_ps = PSM.tile([2 * NS, HW], f32, tag="tr")
            for c in range(4):
                nc.tensor.matmul(
                    nm_ps, textT_s[:, c, :], mwT_s[:, c, :], start=(c == 0), stop=(c == 3)
                )
            mn16 = P.tile([2 * NS, 1], f32)
            nc.vector.tensor_reduce(mn16, nm_ps, axis=AX.X, op=OP.min)
            m016 = P.tile([2 * NS, HW], f32)
            nc.vector.tensor_scalar(
                out=m016, in0=nm_ps, scalar1=mn16[:, 0:1], scalar2=None, op0=OP.subtract
            )
            mx16 = P.tile([2 * NS, 1], f32)
            nc.vector.tensor_reduce(mx16, m016, axis=AX.X, op=OP.max)
            nc.vector.tensor_scalar(
                out=mx16, in0=mx16, scalar1=1e-12, scalar2=None, op0=OP.add
            )
            rmx16 = P.tile([2 * NS, 1], f32)
            nc.vector.reciprocal(rmx16, mx16)
            mask_all = P.tile([2 * NS, HW], f32)
            nc.vector.tensor_scalar(
                out=mask_all, in0=m016, scalar1=rmx16[:, 0:1], scalar2=None, op0=OP.mult
            )
            mT = [
                P.tile([128, 2 * NS], f32, tag="mT1", name="mT1"),
                P.tile([34, 2 * NS], f32, tag="mT2", name="mT2"),
            ]
            nmT = [
                P.tile([128, 2 * NS], f32, tag="nmT1", name="nmT1"),
                P.tile([34, 2 * NS], f32, tag="nmT2", name="nmT2"),
            ]
            for si, (k0, kn) in enumerate(QS):
                t_ps = PSM.tile([128, 2 * NS], f32, tag="tr")
                nc.tensor.transpose(
                    t_ps[0:kn, :], mask_all[:, k0 : k0 + kn], ident[0 : 2 * NS, 0 : 2 * NS]
                )
                nc.scalar.copy(mT[si], t_ps[0:kn, :])
                nc.scalar.mul(nmT[si], t_ps[0:kn, :], -1.0)

            # ---------------- software-pipelined per-sample ----------------
            state = {}

            def phase_a(i):
                """Loads, norms, broadcast row, G matmuls for sample i."""
                s = {}
                s["xq"] = [
                    IO.tile([128, F], f32r, tag="xq1", name="xq1"),
                    IO.tile([34, F], f32r, tag="xq2", name="xq2"),
                ]
                nc.sync.dma_start(out=s["xq"][0], in_=xqT_d[i, 0:128, :])
                nc.sync.dma_start(out=s["xq"][1], in_=xqT_d[i, 128:162, :])
                s["tp"] = [
                    IO.tile([128, 2, F], f32r, tag="tp1", name="tp1"),
                    IO.tile([34, 2, F], f32r, tag="tp2", name="tp2"),
                ]
                nc.sync.dma_start(out=s["tp"][0], in_=tpn_d[i, 0:128, :, :])
                nc.sync.dma_start(out=s["tp"][1], in_=tpn_d[i, 128:162, :, :])
                rhs_t = IO1.tile([128, 16, 324], f32r, tag="rhs")
                nc.sync.dma_start(out=rhs_t, in_=rhs_d[i])
                own_nat = IO1.tile([128, 16, HW], f32r, tag="own_nat")
                nc.sync.dma_start(out=own_nat, in_=xqn_d[i])

                # own norms -> rqT [qn,1] = TEMP / max(||q||, 1e-12)
                s["rqT"] = []
                for si, (q0, qn) in enumerate(QS):
                    xf = s["xq"][si].bitcast(f32)
                    st = WK.tile([128, 4, 6], f32, tag="st")
                    for blk in range(4):
                        nc.vector.bn_stats(
                            out=st[0:qn, blk, :], in_=xf[:, blk * 512 : (blk + 1) * 512]
                        )
                    mv = WK.tile([128, 2], f32, tag="mv")
                    nc.vector.bn_aggr(out=mv[0:qn], in_=st[0:qn])
                    r_ = WK.tile([128, 1], f32, tag=f"rq{si}")
                    nc.vector.tensor_mul(r_[0:qn], mv[0:qn, 0:1], mv[0:qn, 0:1])
                    nc.vector.tensor_add(r_[0:qn], r_[0:qn], mv[0:qn, 1:2])
                    nc.scalar.activation(
                        out=r_[0:qn], in_=r_[0:qn], func=AF.Sqrt, scale=float(F)
                    )
                    nc.vector.tensor_scalar(
                        out=r_[0:qn], in0=r_[0:qn], scalar1=1e-12, scalar2=None, op0=OP.max
                    )
                    nc.vector.reciprocal(r_[0:qn], r_[0:qn])
                    nc.vector.tensor_scalar(
                        out=r_[0:qn], in0=r_[0:qn], scalar1=TEMP, scalar2=None, op0=OP.mult
                    )
                    s["rqT"].append(r_)

                # pos/neg norms -> rowbuf[0:324]; mk row -> rowbuf[324:486]
                rowbuf = WK.tile([1, 486], f32, tag="rowbuf")
                for b in range(2):
                    for si, (k0, kn) in enumerate(QS):
                        src = s["tp"][si][:, b, :].bitcast(f32)
                        nsq = WK.tile([128, 1], f32, tag="nsq")
                        if b == 0:
                            parts = WK.tile([128, 4], f32, tag="nparts")
                            scr = WK.tile([128, 512], f32, tag="sqscr", bufs=1)
                            for blk in range(4):
                                nc.scalar.activation(
                                    out=scr[0:kn],
                                    in_=src[:, blk * 512 : (blk + 1) * 512],
                                    func=AF.Square,
                                    accum_out=parts[0:kn, blk : blk + 1],
                                )
                            nc.vector.tensor_reduce(
                                nsq[0:kn], parts[0:kn], axis=AX.X, op=OP.add
                            )
                        else:
                            st = WK.tile([128, 4, 6], f32, tag="st")
                            for blk in range(4):
                                nc.vector.bn_stats(
                                    out=st[0:kn, blk, :],
                                    in_=src[:, blk * 512 : (blk + 1) * 512],
                                )
                            mv = WK.tile([128, 2], f32, tag="mv")
                            nc.vector.bn_aggr(out=mv[0:kn], in_=st[0:kn])
                            nc.vector.tensor_mul(nsq[0:kn], mv[0:kn, 0:1], mv[0:kn, 0:1])
                            nc.vector.tensor_add(nsq[0:kn], nsq[0:kn], mv[0:kn, 1:2])
                            nc.vector.tensor_scalar(
                                out=nsq[0:kn], in0=nsq[0:kn], scalar1=float(F),
                                scalar2=None, op0=OP.mult,
                            )
                        nc.scalar.activation(out=nsq[0:kn], in_=nsq[0:kn], func=AF.Sqrt)
                        nc.vector.tensor_scalar(
                            out=nsq[0:kn], in0=nsq[0:kn], scalar1=1e-12, scalar2=None,
                            op0=OP.max,
                        )
                        rk = WK.tile([128, 1], f32, tag="rk")
                        nc.vector.reciprocal(rk[0:kn], nsq[0:kn])
                        t_ps = PSM.tile([1, 128], f32, tag="tr")
                        nc.tensor.transpose(
                            t_ps[0:1, 0:kn], rk[0:kn, 0:1], ident[0:kn, 0:kn]
                        )
                        nc.vector.tensor_copy(
                            rowbuf[0:1, b * HW + k0 : b * HW + k0 + kn], t_ps[0:1, 0:kn]
                        )
                for si, (k0, kn) in enumerate(QS):
                    t_ps = PSM.tile([1, 128], f32, tag="tr")
                    nc.tensor.transpose(
                        t_ps[0:1, 0:kn],
                        mT[si][:, NS + i : NS + i + 1],
                        ident[0:kn, 0:kn],
                    )
                    nc.vector.tensor_copy(
                        rowbuf[0:1, 324 + k0 : 324 + k0 + kn], t_ps[0:1, 0:kn]
                    )
                row_dr = DR.tile([1, 486], f32, tag="row_dr")
                nc.gpsimd.dma_start(out=row_dr, in_=rowbuf)
                bc_sb = WK.tile([128, 486], f32, tag="bc_sb")
                nc.gpsimd.dma_start(out=bc_sb, in_=row_dr.to_broadcast((128, 486)))
                s["bc"] = bc_sb

                # G matmuls
                s["g"] = []
                for si, (q0, qn) in enumerate(QS):
                    g_ps = PSM.tile([128, 324], f32, tag="g")
                    for c in range(16):
                        nc.tensor.matmul(
                            g_ps[0:qn],
                            own_nat[:, c, q0 : q0 + qn],
                            rhs_t[:, c, :],
                            start=(c == 0),
                            stop=(c == 15),
                        )
                    s["g"].append(g_ps)
                state[i] = s

            def phase_m(i):
                """Softmax -> scaled E for sample i (DVE/ACT only)."""
                s = state[i]
                s["E"] = []
                s["rs"] = []
                for si, (q0, qn) in enumerate(QS):
                    E = WK.tile([128, 324], f32, tag="E")
                    nc.vector.scalar_tensor_tensor(
                        out=E[0:qn],
                        in0=s["g"][si][0:qn],
                        scalar=s["rqT"][si][0:qn, 0:1],
                        in1=s["bc"][0:qn, 0:324],
                        op0=OP.mult,
                        op1=OP.mult,
                    )
                    nc.scalar.activation(
                        out=E[0:qn], in_=E[0:qn], func=AF.Exp, bias=cm50[0:qn, 0:1]
                    )
                    rs = WK.tile([128, 2], f32, tag="rs")
                    nc.vector.tensor_reduce(
                        rs[0:qn, 0:1], E[0:qn, 0:HW], axis=AX.X, op=OP.add
                    )
                    nc.vector.tensor_reduce(
                        rs[0:qn, 1:2], E[0:qn, HW : 2 * HW], axis=AX.X, op=OP.add
                    )
                    nc.vector.reciprocal(rs[0:qn], rs[0:qn])
                    # fold 1/rowsum into E (per branch)
                    nc.vector.tensor_scalar(
                        out=E[0:qn, 0:HW], in0=E[0:qn, 0:HW],
                        scalar1=rs[0:qn, 0:1], scalar2=None, op0=OP.mult,
                    )
                    nc.vector.tensor_scalar(
                        out=E[0:qn, HW : 2 * HW], in0=E[0:qn, HW : 2 * HW],
                        scalar1=rs[0:qn, 1:2], scalar2=None, op0=OP.mult,
                    )
                    s["E"].append(E)
                    s["rs"].append(rs)

            def phase_b(i):
                """E transposes, warp (psum pre-seeded with -feat), consume."""
                s = state[i]
                recon = [
                    IO1.tile([128, F], f32, tag="rc1", name="rc1"),
                    IO1.tile([34, F], f32, tag="rc2", name="rc2"),
                ]
                rowland = WK.tile([1, 2 * HW], f32, tag="rowland")
                for si, (q0, qn) in enumerate(QS):
                    E = s["E"][si]
                    # comask raw (pos branch, rs already folded into E)
                    cmscr = WK.tile([128, HW], f32, tag="cmscr")
                    cmacc = WK.tile([128, 1], f32, tag="cmacc")
                    nc.gpsimd.tensor_mul(
                        cmscr[0:qn], E[0:qn, 0:HW], s["bc"][0:qn, 324:486]
                    )
                    nc.vector.tensor_reduce(
                        cmacc[0:qn], cmscr[0:qn], axis=AX.X, op=OP.add
                    )
                    pack = WK.tile([128, 4], f32, tag="pack")
                    nc.vector.tensor_scalar(
                        out=pack[0:qn, 2:3], in0=cmacc[0:qn],
                        scalar1=mT[si][:, i : i + 1], scalar2=None, op0=OP.mult,
                    )

                    # E transposes -> warp lhsT (f32r)
                    ET = []
                    for b in range(2):
                        e1 = WK.tile([128, 128], f32r, tag=f"et1b{b}")
                        e2 = WK.tile([34, 128], f32r, tag=f"et2b{b}")
                        for ki, (k0, kn) in enumerate(QS):
                            t_ps = PSM.tile([128, 128], f32, tag="tr")
                            nc.tensor.transpose(
                                t_ps[0:kn, 0:qn],
                                E[0:qn, b * HW + k0 : b * HW + k0 + kn],
                                ident[0:qn, 0:qn],
                            )
                            et = e1 if ki == 0 else e2
                            nc.vector.tensor_copy(et[0:kn, 0:qn], t_ps[0:kn, 0:qn])
                        ET.append((e1, e2))

                    d2acc = WK.tile([128, 8], f32, tag="d2acc")
                    for b in range(2):
                        e1, e2 = ET[b]
                        for fb in range(NB):
                            fsl = slice(fb * 512, (fb + 1) * 512)
                            w_ps = PSW.tile([128, 512], f32, tag="w")
                            # seed with -feat, then accumulate warp
                            nc.tensor.matmul(
                                w_ps[0:qn],
                                negI[0:qn, 0:qn],
                                s["xq"][si][:, fsl],
                                start=True,
                                stop=False,
                            )
                            nc.tensor.matmul(
                                w_ps[0:qn], e1[:, 0:qn], s["tp"][0][:, b, fsl],
                                start=False, stop=False,
                            )
                            nc.tensor.matmul(
                                w_ps[0:qn], e2[:, 0:qn], s["tp"][1][:, b, fsl],
                                start=False, stop=True,
                            )
                            # d^2 partial: sum((eps - mask*(warp-feat))^2)
                            scr = WK.tile([128, 512], f32, tag="sqscr", bufs=1)
                            nc.scalar.activation(
                                out=scr[0:qn],
                                in_=w_ps[0:qn],
                                func=AF.Square,
                                scale=nmT[si][:, i : i + 1],
                                bias=ceps[0:qn, 0:1],
                                accum_out=d2acc[0:qn, b * NB + fb : b * NB + fb + 1],
                            )
                            if b == 0:
                                # recon = mask*(warp-feat) + feat
                                nc.vector.scalar_tensor_tensor(
                                    out=recon[si][:, fsl],
                                    in0=w_ps[0:qn],
                                    scalar=mT[si][:, i : i + 1],
                                    in1=s["xq"][si].bitcast(f32)[:, fsl],
                                    op0=OP.mult,
                                    op1=OP.add,
                                )

                    d2 = WK.tile([128, 2], f32, tag="d2")
                    nc.vector.tensor_reduce(
                        d2[0:qn],
                        d2acc.rearrange("p (b f) -> p b f", b=2)[0:qn],
                        axis=AX.X,
                        op=OP.add,
                    )
                    nc.scalar.activation(out=pack[0:qn, 0:2], in_=d2[0:qn], func=AF.Sqrt)
                    nc.vector.tensor_sub(
                        pack[0:qn, 3:4], pack[0:qn, 0:1], pack[0:qn, 1:2]
                    )
                    nc.vector.tensor_scalar(
                        out=pack[0:qn, 3:4], in0=pack[0:qn, 3:4],
                        scalar1=MARGIN, scalar2=0.0, op0=OP.add, op1=OP.max,
                    )
                    for col, off in ((2, 0), (3, HW)):
                        t_ps = PSM.tile([1, 128], f32, tag="tr")
                        nc.tensor.transpose(
                            t_ps[0:1, 0:qn], pack[0:qn, col : col + 1], ident[0:qn, 0:qn]
                        )
                        nc.vector.tensor_copy(
                            rowland[0:1, off + q0 : off + q0 + qn], t_ps[0:1, 0:qn]
                        )

                mn1 = WK.tile([1, 1], f32, tag="mn1")
                nc.vector.tensor_reduce(mn1, rowland[0:1, 0:HW], axis=AX.X, op=OP.min)
                cmr = WK.tile([1, HW], f32, tag="cmr")
                nc.vector.tensor_scalar(
                    out=cmr, in0=rowland[0:1, 0:HW], scalar1=mn1[0:1, 0:1],
                    scalar2=None, op0=OP.subtract,
                )
                mx1 = WK.tile([1, 1], f32, tag="mx1")
                nc.vector.tensor_reduce(mx1, cmr, axis=AX.X, op=OP.max)
                nc.vector.tensor_scalar(
                    out=mx1, in0=mx1, scalar1=1e-12, scalar2=None, op0=OP.add
                )
                rx1 = WK.tile([1, 1], f32, tag="rx1")
                nc.vector.reciprocal(rx1, mx1)
                nc.vector.tensor_scalar(
                    out=cmr, in0=cmr, scalar1=rx1[0:1, 0:1], scalar2=None, op0=OP.mult
                )
                dscr = WK.tile([1, HW], f32, tag="dscr")
                nc.vector.tensor_mul(dscr, cmr, rowland[0:1, HW : 2 * HW])
                nc.vector.tensor_reduce(
                    dots[0:1, i : i + 1], dscr, axis=AX.X, op=OP.add
                )

                nc.scalar.dma_start(out=reconT_d[i, 0:128, :], in_=recon[0])
                nc.scalar.dma_start(out=reconT_d[i, 128:162, :], in_=recon[1])
                del state[i]

            for k in range(NS + 1):
                if k > 0:
                    phase_m(k - 1)
                if k < NS:
                    phase_a(k)
                if k > 0:
                    phase_b(k - 1)

            lsum = P.tile([1, 1], f32)
            nc.vector.tensor_reduce(lsum, dots, axis=AX.X, op=OP.add)
            nc.sync.dma_start(out=loss_d, in_=lsum)

    return nc


def _get_program():
    if "nc" not in _cache:
        nc = build_program()
        nc.finalize()
        _cache["nc"] = nc
    return _cache["nc"]


def prepare_in_maps(feat_v, feat_t, text, maskW, pos_idx, neg_idx):
    feat_v = np.asarray(feat_v, dtype=np.float32)
    feat_t = np.asarray(feat_t, dtype=np.float32)
    text = np.asarray(text, dtype=np.float32)
    maskW = np.asarray(maskW, dtype=np.float32)
    pos = np.asarray(pos_idx).astype(np.int64)
    neg = np.asarray(neg_idx).astype(np.int64)

    n = 2 * feat_v.shape[0]  # 64
    feat = np.concatenate([feat_v, feat_t], axis=0).reshape(n, F, HW)
    featT = np.ascontiguousarray(feat.transpose(0, 2, 1))  # [64, 162, 2048]
    mwT = np.ascontiguousarray(maskW.T)  # [512, 162]

    in_maps = []
    for c in range(8):
        own = slice(c * NS, (c + 1) * NS)
        po = pos[own]
        ne = neg[own]
        pf = feat[po].reshape(NS, 128, 16, HW)
        nf = feat[ne].reshape(NS, 128, 16, HW)
        rhs = np.ascontiguousarray(
            np.stack([pf, nf], axis=3).reshape(NS, 128, 16, 2 * HW)
        )
        tpn = np.ascontiguousarray(np.stack([featT[po], featT[ne]], axis=2))
        textT = np.ascontiguousarray(
            np.concatenate([text[own], text[po]], axis=0).T
        )
        in_maps.append(
            {
                "xqT": np.ascontiguousarray(featT[own]),
                "xqnat": np.ascontiguousarray(feat[own].reshape(NS, 128, 16, HW)),
                "rhs": rhs,
                "tpn": tpn,
                "textT": textT,
                "mwT": mwT,
            }
        )
    return in_maps, n


def kernel(feat_v, feat_t, text, maskW, pos_idx, neg_idx):
    global LAST_EXEC_NS
    from concourse.bass_utils import run_bass_kernel_spmd

    in_maps, n = prepare_in_maps(feat_v, feat_t, text, maskW, pos_idx, neg_idx)
    nc = _get_program()
    res = run_bass_kernel_spmd(nc, in_maps, list(range(8)), trace=TRACE)
    if TRACE:
        LAST_EXEC_NS = res.exec_time_ns

    recon = np.empty((n, F, HW), dtype=np.float32)
    loss = 0.0
    for c in range(8):
        r = res.results[c]
        recon[c * NS : (c + 1) * NS] = r["reconT"].transpose(0, 2, 1)
        loss += float(r["loss"][0, 0])
    loss = np.float32(loss / (n * HW))
    return recon.reshape(n, F, 18, 9), loss


# revision 35
# speedup vs baseline: 1.1207x; 1.1207x over previous
"""Trainium2 Bass kernel for nn_CMAlign_mask (sparse_attention).

Strategy: data-parallel over the 64-sample batch, 8 samples per NeuronCore.
The cross-sample gathers feat[pos_idx]/feat[neg_idx] are resolved on the host
during sharding (indices are host-visible), so the device program is purely
per-sample. Each sample's feature tensors are shipped in two layouts:
  - natural [f, hw] (f interleaved as 16p+c) for the gram matmul G = x_q^T x_k
  - transposed [hw, f] for the warp matmul (contraction over hw)

Matmuls run as float32r (~1e-4 rel err, ~2x faster than fp32; measured on HW).
The warp PSUM is pre-seeded with -feat via a -Identity matmul so the PSUM ends
up holding warp-feat directly; the softmax denominator is folded into the
transposed probabilities. Per-sample work is software-pipelined (softmax of
sample i-1 | loads+G of sample i | warp+consume of sample i-1) to keep the PE
stream dense.
"""

import sys

if "/opt/trn_rl_repo" not in sys.path:
    sys.path.insert(0, "/opt/trn_rl_repo")

import numpy as np

NS = 8  # samples per core
HW = 162
F = 2048
NB = 4  # f blocks of 512
TEMP = 50.0
MARGIN = 0.3
EPS_PD = 1e-6
QS = [(0, 128), (128, 34)]  # hw splits (partition-dim limit)

TRACE = False  # set by test harness for profiling
LAST_EXEC_NS = None

_cache = {}


def build_program():
    import concourse.bass as bass  # noqa: F401
    import concourse.mybir as mybir
    import concourse.tile as tile
    from concourse import bacc
    from concourse.masks import make_identity

    f32 = mybir.dt.float32
    f32r = mybir.dt.float32r
    AF = mybir.ActivationFunctionType
    OP = mybir.AluOpType
    AX = mybir.AxisListType

    nc = bacc.Bacc("TRN2", target_bir_lowering=False, debug=False, num_devices=8)

    xqT_d = nc.dram_tensor("xqT", [NS, HW, F], f32r, kind="ExternalInput").ap()
    xqn_d = nc.dram_tensor(
        "xqnat", [NS, 128, 16, HW], f32r, kind="ExternalInput"
    ).ap()
    rhs_d = nc.dram_tensor("rhs", [NS, 128, 16, 324], f32r, kind="ExternalInput").ap()
    tpn_d = nc.dram_tensor("tpn", [NS, HW, 2, F], f32r, kind="ExternalInput").ap()
    textT_d = nc.dram_tensor("textT", [512, 2 * NS], f32, kind="ExternalInput").ap()
    mwT_d = nc.dram_tensor("mwT", [512, HW], f32, kind="ExternalInput").ap()
    reconT_d = nc.dram_tensor("reconT", [NS, HW, F], f32, kind="ExternalOutput").ap()
    loss_d = nc.dram_tensor("loss", [1, 1], f32, kind="ExternalOutput").ap()

    with tile.TileContext(nc) as tc:
        with (
            tc.tile_pool(name="persist", bufs=1) as P,
            tc.tile_pool(name="io2", bufs=2) as IO,
            tc.tile_pool(name="io1", bufs=1) as IO1,
            tc.tile_pool(name="wk", bufs=2) as WK,
            tc.tile_pool(name="ps_mm", bufs=2, space="PSUM") as PSM,
            tc.tile_pool(name="ps_w", bufs=3, space="PSUM") as PSW,
            tc.tile_pool(name="dram", bufs=2, space="DRAM") as DR,
        ):
            ident = P.tile([128, 128], f32)
            make_identity(nc, ident)
            negI = P.tile([128, 128], f32r)
            nc.scalar.mul(negI, ident, -1.0)
            cm50 = P.tile([128, 1], f32)
            nc.vector.memset(cm50, -TEMP)
            ceps = P.tile([128, 1], f32)
            nc.vector.memset(ceps, EPS_PD)
            dots = P.tile([1, NS], f32)

            # ---------------- mask preamble ----------------
            textT_s = P.tile([128, 4, 2 * NS], f32)
            nc.sync.dma_start(
                out=textT_s, in_=textT_d.rearrange("(c p) j -> p c j", p=128)
            )
            mwT_s = P.tile([128, 4, HW], f32)
            nc.sync.dma_start(out=mwT_s, in_=mwT_d.rearrange("(c p) s -> p c s", p=128))
            nm_ps = PSM.tile([2 * NS, HW], f32, tag="tr")
            for c in range(4):
                nc.tensor.matmul(
                    nm_ps, textT_s[:, c, :], mwT_s[:, c, :], start=(c == 0), stop=(c == 3)
                )
            mn16 = P.tile([2 * NS, 1], f32)
            nc.vector.tensor_reduce(mn16, nm_ps, axis=AX.X, op=OP.min)
            m016 = P.tile([2 * NS, HW], f32)
            nc.vector.tensor_scalar(
                out=m016, in0=nm_ps, scalar1=mn16[:, 0:1], scalar2=None, op0=OP.subtract
            )
            mx16 = P.tile([2 * NS, 1], f32)
            nc.vector.tensor_reduce(mx16, m016, axis=AX.X, op=OP.max)
            nc.vector.tensor_scalar(
                out=mx16, in0=mx16, scalar1=1e-12, scalar2=None, op0=OP.add
            )
            rmx16 = P.tile([2 * NS, 1], f32)
            nc.vector.reciprocal(rmx16, mx16)
            mask_all = P.tile([2 * NS, HW], f32)
            nc.vector.tensor_scalar(
                out=mask_all, in0=m016, scalar1=rmx16[:, 0:1], scalar2=None, op0=OP.mult
            )
            mT = [
                P.tile([128, 2 * NS], f32, tag="mT1", name="mT1"),
                P.tile([34, 2 * NS], f32, tag="mT2", name="mT2"),
            ]
            nmT = [
                P.tile([128, 2 * NS], f32, tag="nmT1", name="nmT1"),
                P.tile([34, 2 * NS], f32, tag="nmT2", name="nmT2"),
            ]
            for si, (k0, kn) in enumerate(QS):
                t_ps = PSM.tile([128, 2 * NS], f32, tag="tr")
                nc.tensor.transpose(
                    t_ps[0:kn, :], mask_all[:, k0 : k0 + kn], ident[0 : 2 * NS, 0 : 2 * NS]
                )
                nc.scalar.copy(mT[si], t_ps[0:kn, :])
                nc.scalar.mul(nmT[si], t_ps[0:kn, :], -1.0)

            # ---------------- software-pipelined per-sample ----------------
            state = {}

            def phase_a1(i):
                """Loads, G matmuls, own norms for sample i."""
                s = {}
                s["xq"] = [
                    IO.tile([128, F], f32r, tag="xq1", name="xq1"),
                    IO.tile([34, F], f32r, tag="xq2", name="xq2"),
                ]
                nc.sync.dma_start(out=s["xq"][0], in_=xqT_d[i, 0:128, :])
                nc.sync.dma_start(out=s["xq"][1], in_=xqT_d[i, 128:162, :])
                s["tp"] = [
                    IO.tile([128, 2, F], f32r, tag="tp1", name="tp1"),
                    IO.tile([34, 2, F], f32r, tag="tp2", name="tp2"),
                ]
                nc.sync.dma_start(out=s["tp"][0], in_=tpn_d[i, 0:128, :, :])
                nc.sync.dma_start(out=s["tp"][1], in_=tpn_d[i, 128:162, :, :])
                rhs_t = IO1.tile([128, 16, 324], f32r, tag="rhs")
                nc.sync.dma_start(out=rhs_t, in_=rhs_d[i])
                own_nat = IO1.tile([128, 16, HW], f32r, tag="own_nat")
                nc.sync.dma_start(out=own_nat, in_=xqn_d[i])

                # G matmuls (first in the PE stream for this iteration)
                s["g"] = []
                for si, (q0, qn) in enumerate(QS):
                    g_ps = PSM.tile([128, 324], f32, tag="g")
                    for c in range(16):
                        nc.tensor.matmul(
                            g_ps[0:qn],
                            own_nat[:, c, q0 : q0 + qn],
                            rhs_t[:, c, :],
                            start=(c == 0),
                            stop=(c == 15),
                        )
                    s["g"].append(g_ps)

                # own norms -> rqT [qn,1] = TEMP / max(||q||, 1e-12)
                s["rqT"] = []
                for si, (q0, qn) in enumerate(QS):
                    xf = s["xq"][si].bitcast(f32)
                    st = WK.tile([128, 4, 6], f32, tag="st")
                    for blk in range(4):
                        nc.vector.bn_stats(
                            out=st[0:qn, blk, :], in_=xf[:, blk * 512 : (blk + 1) * 512]
                        )
                    mv = WK.tile([128, 2], f32, tag="mv")
                    nc.vector.bn_aggr(out=mv[0:qn], in_=st[0:qn])
                    r_ = WK.tile([128, 1], f32, tag=f"rq{si}")
                    nc.vector.tensor_mul(r_[0:qn], mv[0:qn, 0:1], mv[0:qn, 0:1])
                    nc.vector.tensor_add(r_[0:qn], r_[0:qn], mv[0:qn, 1:2])
                    nc.scalar.activation(
                        out=r_[0:qn], in_=r_[0:qn], func=AF.Sqrt, scale=float(F)
                    )
                    nc.vector.tensor_scalar(
                        out=r_[0:qn], in0=r_[0:qn], scalar1=1e-12, scalar2=None, op0=OP.max
                    )
                    nc.vector.reciprocal(r_[0:qn], r_[0:qn])
                    nc.vector.tensor_scalar(
                        out=r_[0:qn], in0=r_[0:qn], scalar1=TEMP, scalar2=None, op0=OP.mult
                    )
                    s["rqT"].append(r_)

                state[i] = s

            def phase_a2(i):
                """pos/neg norms, rnk/mk rows, broadcast (late PE transposes)."""
                s = state[i]
                # pos/neg norms -> rowbuf[0:324]; mk row -> rowbuf[324:486]
                rowbuf = WK.tile([1, 486], f32, tag="rowbuf")
                for b in range(2):
                    for si, (k0, kn) in enumerate(QS):
                        src = s["tp"][si][:, b, :].bitcast(f32)
                        nsq = WK.tile([128, 1], f32, tag="nsq")
                        if b == 0:
                            parts = WK.tile([128, 4], f32, tag="nparts")
                            scr = WK.tile([128, 512], f32, tag="sqscr", bufs=1)
                            for blk in range(4):
                                nc.scalar.activation(
                                    out=scr[0:kn],
                                    in_=src[:, blk * 512 : (blk + 1) * 512],
                                    func=AF.Square,
                                    accum_out=parts[0:kn, blk : blk + 1],
                                )
                            nc.vector.tensor_reduce(
                                nsq[0:kn], parts[0:kn], axis=AX.X, op=OP.add
                            )
                        else:
                            st = WK.tile([128, 4, 6], f32, tag="st")
                            for blk in range(4):
                                nc.vector.bn_stats(
                                    out=st[0:kn, blk, :],
                                    in_=src[:, blk * 512 : (blk + 1) * 512],
                                )
                            mv = WK.tile([128, 2], f32, tag="mv")
                            nc.vector.bn_aggr(out=mv[0:kn], in_=st[0:kn])
                            nc.vector.tensor_mul(nsq[0:kn], mv[0:kn, 0:1], mv[0:kn, 0:1])
                            nc.vector.tensor_add(nsq[0:kn], nsq[0:kn], mv[0:kn, 1:2])
                            nc.vector.tensor_scalar(
                                out=nsq[0:kn], in0=nsq[0:kn], scalar1=float(F),
                                scalar2=None, op0=OP.mult,
                            )
                        nc.scalar.activation(out=nsq[0:kn], in_=nsq[0:kn], func=AF.Sqrt)
                        nc.vector.tensor_scalar(
                            out=nsq[0:kn], in0=nsq[0:kn], scalar1=1e-12, scalar2=None,
                            op0=OP.max,
                        )
                        rk = WK.tile([128, 1], f32, tag="rk")
                        nc.vector.reciprocal(rk[0:kn], nsq[0:kn])
                        t_ps = PSM.tile([1, 128], f32, tag="tr")
                        nc.tensor.transpose(
                            t_ps[0:1, 0:kn], rk[0:kn, 0:1], ident[0:kn, 0:kn]
                        )
                        nc.vector.tensor_copy(
                            rowbuf[0:1, b * HW + k0 : b * HW + k0 + kn], t_ps[0:1, 0:kn]
                        )
                for si, (k0, kn) in enumerate(QS):
                    t_ps = PSM.tile([1, 128], f32, tag="tr")
                    nc.tensor.transpose(
                        t_ps[0:1, 0:kn],
                        mT[si][:, NS + i : NS + i + 1],
                        ident[0:kn, 0:kn],
                    )
                    nc.vector.tensor_copy(
                        rowbuf[0:1, 324 + k0 : 324 + k0 + kn], t_ps[0:1, 0:kn]
                    )
                row_dr = DR.tile([1, 486], f32, tag="row_dr")
                nc.gpsimd.dma_start(out=row_dr, in_=rowbuf)
                bc_sb = WK.tile([128, 486], f32, tag="bc_sb")
                nc.gpsimd.dma_start(out=bc_sb, in_=row_dr.to_broadcast((128, 486)))
                s["bc"] = bc_sb

            def phase_m(i):
                """Softmax -> scaled E for sample i (DVE/ACT only)."""
                s = state[i]
                s["E"] = []
                s["rs"] = []
                for si, (q0, qn) in enumerate(QS):
                    E = WK.tile([128, 324], f32, tag="E")
                    nc.vector.scalar_tensor_tensor(
                        out=E[0:qn],
                        in0=s["g"][si][0:qn],
                        scalar=s["rqT"][si][0:qn, 0:1],
                        in1=s["bc"][0:qn, 0:324],
                        op0=OP.mult,
                        op1=OP.mult,
                    )
                    nc.scalar.activation(
                        out=E[0:qn], in_=E[0:qn], func=AF.Exp, bias=cm50[0:qn, 0:1]
                    )
                    rs = WK.tile([128, 2], f32, tag="rs")
                    nc.vector.tensor_reduce(
                        rs[0:qn, 0:1], E[0:qn, 0:HW], axis=AX.X, op=OP.add
                    )
                    nc.vector.tensor_reduce(
                        rs[0:qn, 1:2], E[0:qn, HW : 2 * HW], axis=AX.X, op=OP.add
                    )
                    nc.vector.reciprocal(rs[0:qn], rs[0:qn])
                    # fold 1/rowsum into E (per branch)
                    nc.vector.tensor_scalar(
                        out=E[0:qn, 0:HW], in0=E[0:qn, 0:HW],
                        scalar1=rs[0:qn, 0:1], scalar2=None, op0=OP.mult,
                    )
                    nc.vector.tensor_scalar(
                        out=E[0:qn, HW : 2 * HW], in0=E[0:qn, HW : 2 * HW],
                        scalar1=rs[0:qn, 1:2], scalar2=None, op0=OP.mult,
                    )
                    s["E"].append(E)
                    s["rs"].append(rs)

            def phase_b(i):
                """E transposes, warp (psum pre-seeded with -feat), consume."""
                s = state[i]
                recon = [
                    IO1.tile([128, F], f32, tag="rc1", name="rc1"),
                    IO1.tile([34, F], f32, tag="rc2", name="rc2"),
                ]
                rowland = WK.tile([1, 2 * HW], f32, tag="rowland")
                for si, (q0, qn) in enumerate(QS):
                    E = s["E"][si]
                    # comask raw (pos branch, rs already folded into E)
                    cmscr = WK.tile([128, HW], f32, tag="cmscr")
                    cmacc = WK.tile([128, 1], f32, tag="cmacc")
                    nc.gpsimd.tensor_mul(
                        cmscr[0:qn], E[0:qn, 0:HW], s["bc"][0:qn, 324:486]
                    )
                    nc.vector.tensor_reduce(
                        cmacc[0:qn], cmscr[0:qn], axis=AX.X, op=OP.add
                    )
                    pack = WK.tile([128, 4], f32, tag="pack")
                    nc.vector.tensor_scalar(
                        out=pack[0:qn, 2:3], in0=cmacc[0:qn],
                        scalar1=mT[si][:, i : i + 1], scalar2=None, op0=OP.mult,
                    )

                    # E transposes -> warp lhsT (f32r)
                    ET = []
                    for b in range(2):
                        e1 = WK.tile([128, 128], f32r, tag=f"et1b{b}")
                        e2 = WK.tile([34, 128], f32r, tag=f"et2b{b}")
                        for ki, (k0, kn) in enumerate(QS):
                            t_ps = PSM.tile([128, 128], f32, tag="tr")
                            nc.tensor.transpose(
                                t_ps[0:kn, 0:qn],
                                E[0:qn, b * HW + k0 : b * HW + k0 + kn],
                                ident[0:qn, 0:qn],
                            )
                            et = e1 if ki == 0 else e2
                            nc.vector.tensor_copy(et[0:kn, 0:qn], t_ps[0:kn, 0:qn])
                        ET.append((e1, e2))

                    d2acc = WK.tile([128, 8], f32, tag="d2acc")
                    for b in range(2):
                        e1, e2 = ET[b]
                        for fb in range(NB):
                            fsl = slice(fb * 512, (fb + 1) * 512)
                            w_ps = PSW.tile([128, 512], f32, tag="w")
                            # seed with -feat, then accumulate warp
                            nc.tensor.matmul(
                                w_ps[0:qn],
                                negI[0:qn, 0:qn],
                                s["xq"][si][:, fsl],
                                start=True,
                                stop=False,
                            )
                            nc.tensor.matmul(
                                w_ps[0:qn], e1[:, 0:qn], s["tp"][0][:, b, fsl],
                                start=False, stop=False,
                            )
                            nc.tensor.matmul(
                                w_ps[0:qn], e2[:, 0:qn], s["tp"][1][:, b, fsl],
                                start=False, stop=True,
                            )
                            # d^2 partial: sum((eps - mask*(warp-feat))^2)
                            scr = WK.tile([128, 512], f32, tag="sqscr", bufs=1)
                            nc.scalar.activation(
                                out=scr[0:qn],
                                in_=w_ps[0:qn],
                                func=AF.Square,
                                scale=nmT[si][:, i : i + 1],
                                bias=ceps[0:qn, 0:1],
                                accum_out=d2acc[0:qn, b * NB + fb : b * NB + fb + 1],
                            )
                            if b == 0:
                                # recon = mask*(warp-feat) + feat
                                nc.vector.scalar_tensor_tensor(
                                    out=recon[si][:, fsl],
                                    in0=w_ps[0:qn],
                                    scalar=mT[si][:, i : i + 1],
                                    in1=s["xq"][si].bitcast(f32)[:, fsl],
                                    op0=OP.mult,
                                    op1=OP.add,
                                )

                    d2 = WK.tile([128, 2], f32, tag="d2")
                    nc.vector.tensor_reduce(
                        d2[0:qn],
                        d2acc.rearrange("p (b f) -> p b f", b=2)[0:qn],
                        axis=AX.X,
                        op=OP.add,
                    )
                    nc.scalar.activation(out=pack[0:qn, 0:2], in_=d2[0:qn], func=AF.Sqrt)
                    nc.vector.tensor_sub(
                        pack[0:qn, 3:4], pack[0:qn, 0:1], pack[0:qn, 1:2]
                    )
                    nc.vector.tensor_scalar(
                        out=pack[0:qn, 3:4], in0=pack[0:qn, 3:4],
                        scalar1=MARGIN, scalar2=0.0, op0=OP.add, op1=OP.max,
                    )
                    for col, off in ((2, 0), (3, HW)):
                        t_ps = PSM.tile([1, 128], f32, tag="tr")
                        nc.tensor.transpose(
                            t_ps[0:1, 0:qn], pack[0:qn, col : col + 1], ident[0:qn, 0:qn]
                        )
                        nc.vector.tensor_copy(
                            rowland[0:1, off + q0 : off + q0 + qn], t_ps[0:1, 0:qn]
                        )

                mn1 = WK.tile([1, 1], f32, tag="mn1")
                nc.vector.tensor_reduce(mn1, rowland[0:1, 0:HW], axis=AX.X, op=OP.min)
                cmr = WK.tile([1, HW], f32, tag="cmr")
                nc.vector.tensor_scalar(
                    out=cmr, in0=rowland[0:1, 0:HW], scalar1=mn1[0:1, 0:1],
                    scalar2=None, op0=OP.subtract,
                )
                mx1 = WK.tile([1, 1], f32, tag="mx1")
                nc.vector.tensor_reduce(mx1, cmr, axis=AX.X, op=OP.max)
                nc.vector.tensor_scalar(
                    out=mx1, in0=mx1, scalar1=1e-12, scalar2=None, op0=OP.add
                )
                rx1 = WK.tile([1, 1], f32, tag="rx1")
                nc.vector.reciprocal(rx1, mx1)
                nc.vector.tensor_scalar(
                    out=cmr, in0=cmr, scalar1=rx1[0:1, 0:1], scalar2=None, op0=OP.mult
                )
                dscr = WK.tile([1, HW], f32, tag="dscr")
                nc.vector.tensor_mul(dscr, cmr, rowland[0:1, HW : 2 * HW])
                nc.vector.tensor_reduce(
                    dots[0:1, i : i + 1], dscr, axis=AX.X, op=OP.add
                )

                nc.gpsimd.dma_start(out=reconT_d[i, 0:128, :], in_=recon[0])
                nc.gpsimd.dma_start(out=reconT_d[i, 128:162, :], in_=recon[1])
                del state[i]

            for k in range(NS + 1):
                if k < NS:
                    phase_a1(k)
                if k > 0:
                    phase_m(k - 1)
                if k > 0:
                    phase_b(k - 1)
                if k < NS:
                    phase_a2(k)

            lsum = P.tile([1, 1], f32)
            nc.vector.tensor_reduce(lsum, dots, axis=AX.X, op=OP.add)
            nc.sync.dma_start(out=loss_d, in_=lsum)

    return nc


def _get_program():
    if "nc" not in _cache:
        nc = build_program()
        nc.finalize()
        _cache["nc"] = nc
    return _cache["nc"]


def prepare_in_maps(feat_v, feat_t, text, maskW, pos_idx, neg_idx):
    feat_v = np.asarray(feat_v, dtype=np.float32)
    feat_t = np.asarray(feat_t, dtype=np.float32)
    text = np.asarray(text, dtype=np.float32)
    maskW = np.asarray(maskW, dtype=np.float32)
    pos = np.asarray(pos_idx).astype(np.int64)
    neg = np.asarray(neg_idx).astype(np.int64)

    n = 2 * feat_v.shape[0]  # 64
    feat = np.concatenate([feat_v, feat_t], axis=0).reshape(n, F, HW)
    featT = np.ascontiguousarray(feat.transpose(0, 2, 1))  # [64, 162, 2048]
    mwT = np.ascontiguousarray(maskW.T)  # [512, 162]

    in_maps = []
    for c in range(8):
        own = slice(c * NS, (c + 1) * NS)
        po = pos[own]
        ne = neg[own]
        pf = feat[po].reshape(NS, 128, 16, HW)
        nf = feat[ne].reshape(NS, 128, 16, HW)
        rhs = np.ascontiguousarray(
            np.stack([pf, nf], axis=3).reshape(NS, 128, 16, 2 * HW)
        )
        tpn = np.ascontiguousarray(np.stack([featT[po], featT[ne]], axis=2))
        textT = np.ascontiguousarray(
            np.concatenate([text[own], text[po]], axis=0).T
        )
        in_maps.append(
            {
                "xqT": np.ascontiguousarray(featT[own]),
                "xqnat": np.ascontiguousarray(feat[own].reshape(NS, 128, 16, HW)),
                "rhs": rhs,
                "tpn": tpn,
                "textT": textT,
                "mwT": mwT,
            }
        )
    return in_maps, n


def kernel(feat_v, feat_t, text, maskW, pos_idx, neg_idx):
    global LAST_EXEC_NS
    from concourse.bass_utils import run_bass_kernel_spmd

    in_maps, n = prepare_in_maps(feat_v, feat_t, text, maskW, pos_idx, neg_idx)
    nc = _get_program()
    res = run_bass_kernel_spmd(nc, in_maps, list(range(8)), trace=TRACE)
    if TRACE:
        LAST_EXEC_NS = res.exec_time_ns

    recon = np.empty((n, F, HW), dtype=np.float32)
    loss = 0.0
    for c in range(8):
        r = res.results[c]
        recon[c * NS : (c + 1) * NS] = r["reconT"].transpose(0, 2, 1)
        loss += float(r["loss"][0, 0])
    loss = np.float32(loss / (n * HW))
    return recon.reshape(n, F, 18, 9), loss
